# revision 34
# baseline (speedup 1.0000x reference)
"""Longformer-style blocked local+global attention on 8 Trainium2 NeuronCores.

Problem (hardcoded): B=2, S=4096, D=1024, H=16, DH=64, W=256 (block size =
one-sided window radius), G=64 global prefix tokens.

Sharding: batch x head-group. Core c handles batch b = c//4 and heads
[(c%4)*4, (c%4)*4+4). Everything for one (b, head-group) is independent, so
there are no collectives; the only cross-core interaction is the final
output-projection partial sum, which the host performs (4 partials per batch).

v2 layout strategy (v1 ran projections and attention as two serial phases;
the trace showed the scalar engine idle for the whole projection phase and
the PE half-idle during attention):
  - Projections and attention are software-pipelined at block granularity:
    after the chunk-c projection (512 seq cols), the attention blocks whose
    q/k/v dependencies are satisfied are emitted before chunk c+1.
  - Scores for one (head, block) are packed into a [128,1024] 2-bank PSUM
    tile (4 strip chunks) plus a [128,512] tile (edge chunks + global), so
    softmax needs only TWO exp instructions per head-block (the activation
    engine's cost is ~(cols+352)/1.2GHz regardless of partition count, so
    fewer/wider instructions cut its busy time ~30%).
  - Scores are computed transposed (sT = [key_pos, query]) so exp'd scores
    feed the PV matmul directly as the moving operand; the two heads of a
    head-pair occupy partition rows 0:64 / 64:128 of qT/kT and their score
    matmuls are interleaved per-chunk so disjoint PE row groups stream
    concurrently.
  - v natural layout [S, DH] per head with a 65th ones column so the PV
    matmul accumulates the softmax denominator as row 64 for free.
  - The PV-output PSUM tile is double-buffered and the normalize chain is
    merged to one copy/recip/broadcast per (head-pair, block).
  - exp without max subtraction (scores are O(1) by construction); masked
    entries are zeroed via precomputed 0/1 mask tiles after exp; PSUM
    garbage in never-read score regions is exp'd harmlessly.
  - All matmul inputs bf16; accumulation fp32 in PSUM.
"""

import numpy as np
import ml_dtypes

import concourse.bacc as bacc
import concourse.bass as bass
import concourse.mybir as mybir
import concourse.tile as tile
from concourse.bass_utils import run_bass_kernel_spmd

BF16 = mybir.dt.bfloat16
F32 = mybir.dt.float32
NPBF = ml_dtypes.bfloat16

B, S, D = 2, 4096, 1024
H, DH = 16, 64
W = 256          # block size == window radius
G = 64           # global prefix tokens
NB = S // W      # 16 blocks
SCALE = 1.0 / 8.0  # 1/sqrt(DH)

N_CORES = 8
HEADS_PER_CORE = 4
ECOLS = HEADS_PER_CORE * DH   # 256 embedding columns per core

# mask stack indices (each [128, 512] left-aligned, see build_masks)
M_L1, M_R0, M_EGEN, M_EN1, M_GC, M_R0R1, M_L0L1 = range(7)

# module-level caches
_BUILT = {}
LAST_RESULTS = None


def build_masks():
    """[7, 128, 512] bf16 0/1 masks, left-aligned.

    Local-strip chunk c of query block n holds key rows kj of block n-1
    (c=0,1), n (c=2,3), n+1 (c=4,5). Triangle masks (r = row within chunk,
    q = query within block): c0: q<=r (only q<128 possible); c1: q<=128+r;
    c4: q>=r; c5: q>=128+r (only q>=128 possible).
    """
    r = np.arange(128)[:, None]
    q = np.arange(256)[None, :]
    L0 = (q <= r).astype(np.float32)          # use cols 0:128
    L1 = (q <= 128 + r).astype(np.float32)
    R0 = (q >= r).astype(np.float32)
    R1 = (q >= 128 + r).astype(np.float32)    # use cols 128:256
    L0g = L0 * (r >= G)                       # left-upper chunk w/ global cut
    Gc = np.broadcast_to((r >= G).astype(np.float32), (128, 256)).copy()

    L0h, L0gh, R1h = L0[:, 0:128], L0g[:, 0:128], R1[:, 128:256]

    def pad(*parts):
        m = np.concatenate(parts, axis=1)
        if m.shape[1] < 512:
            m = np.concatenate(
                [m, np.zeros((128, 512 - m.shape[1]), np.float32)], axis=1)
        return m

    m = np.stack([
        pad(L1),             # M_L1
        pad(R0),             # M_R0
        pad(L0h, R1h),       # M_EGEN  [c0|c5]
        pad(L0gh, R1h),      # M_EN1   [c0 w/ global cut|c5] (n==1)
        pad(Gc),             # M_GC    (n==0 c2)
        pad(R0, R1h),        # M_R0R1  (n==0 [c4|c5])
        pad(L0h, L1),        # M_L0L1  (n==15 [c0|c1])
    ]).astype(NPBF)
    return m


def _block_spec(n):
    """Score layout for query block n.

    t01: chunk parts in the [128,1024] PSUM tile, t2: parts in the [128,512]
    tile ('glb' = global-prefix scores, 64 rows). Each part is
    (c, col0, width, q0): chunk c (seq tile 2*(n-1)+c) lands at cols
    [col0, col0+width) covering queries [q0, q0+width). No part may cross a
    512-col PSUM bank boundary. exps are (col_lo, col_hi, rows); masks are
    (col_lo, col_hi, mask_idx) applied to the merged et tile (t2 region at
    +1024); pv lists (c, et_off, width, q0) with 'glb' appended last.
    """
    if n == 0:
        return dict(
            parts=[(2, 0, 256, 0), (3, 256, 256, 0),
                   (4, 512, 256, 0), (5, 768, 128, 128),
                   ("glb", 1024, 256, 0)],
            exp=(0, 1280, 128),
            # masks are (col_lo, col_hi, idx, mask_col_offset), trimmed to
            # the triangular sub-ranges that actually contain invalid entries
            masks=[(0, 256, M_GC, 0), (512, 640, M_R0, 0),
                   (768, 896, M_R0R1, 256)],
            # deps-latest-first: c4 covers the last mask, then chunks whose
            # waits are already implied, so the PE queue streams waitlessly
            pv=[(4, 512, 256, 0), (5, 768, 128, 128),
                (3, 256, 256, 0), (2, 0, 256, 0)],
            glb_off=1024,
        )
    if n == NB - 1:
        # gap at cols 384:512 keeps c2 from crossing the bank boundary
        return dict(
            parts=[(0, 0, 128, 0), (1, 128, 256, 0),
                   (2, 512, 256, 0), (3, 768, 256, 0),
                   ("glb", 1024, 256, 0)],
            exp=(0, 1280, 128),
            masks=[(0, 128, M_L0L1, 0), (256, 384, M_L0L1, 256)],
            pv=[(0, 0, 128, 0), (1, 128, 256, 0),
                (3, 768, 256, 0), (2, 512, 256, 0)],
            glb_off=1024,
        )
    return dict(
        parts=[(1, 0, 256, 0), (2, 256, 256, 0),
               (3, 512, 256, 0), (4, 768, 256, 0),
               (0, 1024, 128, 0), (5, 1152, 128, 128),
               ("glb", 1280, 256, 0)],
        exp=(0, 1536, 128),
        masks=[(128, 256, M_L1, 128), (768, 896, M_R0, 0),
               (1024, 1280, M_EN1 if n == 1 else M_EGEN, 0)],
        pv=[(0, 1024, 128, 0), (5, 1152, 128, 128), (4, 768, 256, 0),
            (1, 0, 256, 0), (3, 512, 256, 0), (2, 256, 256, 0)],
        glb_off=1280,
    )


def build():
    """Build the per-core Bass/Tile program (identical on all 8 cores)."""
    nc = bacc.Bacc("TRN2", target_bir_lowering=False, debug=False)

    xT = nc.dram_tensor("xT", [D, S], BF16, kind="ExternalInput")
    wq = nc.dram_tensor("wq", [D, ECOLS], BF16, kind="ExternalInput")
    wk = nc.dram_tensor("wk", [D, ECOLS], BF16, kind="ExternalInput")
    wv = nc.dram_tensor("wv", [D, ECOLS], BF16, kind="ExternalInput")
    wo = nc.dram_tensor("wo", [ECOLS, D], BF16, kind="ExternalInput")
    masks = nc.dram_tensor("masks", [7, 128, 512], BF16, kind="ExternalInput")
    y = nc.dram_tensor("y", [S, D], BF16, kind="ExternalOutput")

    EXP = mybir.ActivationFunctionType.Exp

    with tile.TileContext(nc) as tc:
        with (
            tc.tile_pool(name="const", bufs=1) as constp,
            tc.tile_pool(name="persist", bufs=1) as pers,
            tc.tile_pool(name="etp", bufs=6) as etp,
            tc.tile_pool(name="attnp", bufs=4) as atp,
            tc.tile_pool(name="smallp", bufs=8) as smp,
            tc.tile_pool(name="yp", bufs=2) as yp,
            tc.tile_pool(name="xstream", bufs=2) as xp,
            tc.tile_pool(name="psA", bufs=2, space="PSUM") as psA,
            tc.tile_pool(name="psC", bufs=2, space="PSUM") as psC,
        ):
            # ---- constants ----
            wq_sb = constp.tile([128, 8, ECOLS], BF16, name="wq_sb")
            wk_sb = constp.tile([128, 8, ECOLS], BF16, name="wk_sb")
            wv_sb = constp.tile([128, 8, ECOLS], BF16, name="wv_sb")
            wo_sb = constp.tile([128, 2, D], BF16, name="wo_sb")
            mk_sb = constp.tile([128, 7, 512], BF16, name="mk_sb")
            for k in range(8):  # wq per-k so the first matmuls start early;
                # gpsimd queue so they run parallel to sync's xT loads
                nc.gpsimd.dma_start(
                    out=wq_sb[:, k, :],
                    in_=wq.ap()[k * 128:(k + 1) * 128, :])
            nc.gpsimd.dma_start(
                out=wk_sb[:], in_=wk.ap().rearrange("(k p) e -> p k e", p=128))
            nc.gpsimd.dma_start(
                out=wv_sb[:], in_=wv.ap().rearrange("(k p) e -> p k e", p=128))
            nc.gpsimd.dma_start(
                out=wo_sb[:], in_=wo.ap().rearrange("(e p) d -> p e d", p=128))
            nc.gpsimd.dma_start(
                out=mk_sb[:], in_=masks.ap().rearrange("m p q -> p m q"))

            # ---- persistent per-head tensors ----
            qT = [pers.tile([128, S], BF16, name=f"qT{i}") for i in range(2)]
            kT = [pers.tile([128, S], BF16, name=f"kT{i}") for i in range(2)]
            # v natural layout: [128 seq-part, 32 seq-tiles, 4 heads, 128]
            # (col 64 = ones for the denominator row; cols 65:128 zero-padded
            # so the PV stationary is a full 128 columns -> FWL-eligible, which
            # lets the PE hide LDWEIGHTS behind the previous matmul; 65-wide
            # stationaries measured +110ns serialized LDW per PV matmul)
            vv = pers.tile([128, S // 128, HEADS_PER_CORE, 128], BF16,
                           name="vv")
            nc.vector.memset(vv[:, :, :, 65:128], 0.0)
            for h in range(HEADS_PER_CORE):
                nc.vector.memset(vv[:, :, h, 64:65], 1.0)

            def emit_proj(c):
                """Q/K/V projections for sequence chunk c (512 cols)."""
                xt = xp.tile([128, 8, 512], BF16, name="xt")
                nc.sync.dma_start(
                    out=xt[:],
                    in_=xT.ap()[:, c * 512:(c + 1) * 512]
                        .rearrange("(k p) s -> p k s", p=128))
                for hp in range(2):
                    ps = psA.tile([128, 1536], F32, name="ps_qk", tag="a")
                    for wsb, off in ((wq_sb, 0), (wk_sb, 512)):
                        for k in range(8):
                            nc.tensor.matmul(
                                ps[:, off:off + 512],
                                wsb[:, k, hp * 128:(hp + 1) * 128],
                                xt[:, k, :],
                                start=(k == 0), stop=(k == 7))
                    nc.vector.tensor_copy(
                        qT[hp][:, c * 512:(c + 1) * 512], ps[:, 0:512])
                    nc.vector.tensor_copy(
                        kT[hp][:, c * 512:(c + 1) * 512], ps[:, 512:1024])
                for half in range(2):  # two [128-seq] subtile pairs -> v
                    ps = psC.tile([128, 512], F32, name="ps_v", tag="c")
                    for i in range(2):
                        ss = half * 2 + i
                        for k in range(8):
                            nc.tensor.matmul(
                                ps[:, i * 256:(i + 1) * 256],
                                xt[:, k, ss * 128:(ss + 1) * 128],
                                wv_sb[:, k, :],
                                start=(k == 0), stop=(k == 7))
                    t0 = c * 4 + half * 2
                    nc.vector.tensor_copy(
                        vv[:, t0:t0 + 2, :, 0:64],
                        ps[:].rearrange("p (s h e) -> p s h e", s=2, h=4))

            def emit_wo_half(n, at_blk, ss):
                """Output projection for 128 of block n's rows. py lives in
                the 1024-wide pool: its recycled slot was consumed an entire
                block ago, so Wo never waits on this block's fresh exps and
                can fill the PE while the scalar engine works."""
                ysb = yp.tile([128, D], BF16, name="ysb")
                for dk in range(2):
                    py_ = psC.tile([128, 512], F32, name="py", tag="c")
                    for e in range(2):
                        nc.tensor.matmul(
                            py_[:],
                            at_blk[:, e, ss * 128:(ss + 1) * 128],
                            wo_sb[:, e, dk * 512:(dk + 1) * 512],
                            start=(e == 0), stop=(e == 1))
                    if dk == 0:
                        nc.vector.tensor_copy(ysb[:, 0:512], py_[:])
                    else:
                        nc.scalar.copy(ysb[:, 512:1024], py_[:])
                r0 = n * 256 + ss * 128
                nc.sync.dma_start(out=y.ap()[r0:r0 + 128, :], in_=ysb[:])

            def emit_attn(n, at_blk, pending):
                """Attention for query block n -> at_blk (normalized).

                Emission order: scores-hp0, Wo-half, scores-hp1, Wo-half,
                PV-hp0, PV-hp1 -- the previous block's Wo matmuls fill the
                PE pipeline during this block's exp latency.
                """
                sp = _block_spec(n)
                ets_all = []
                for hp in range(2):
                    qpair, kpair = qT[hp], kT[hp]
                    # et: [128, 1536] per head: cols 0:1024 = t01 chunks,
                    # 1024:1536 = t2 chunks + global, mirroring psum packing
                    ets = [etp.tile([128, 1536], BF16, name="et")
                           for _ in range(2)]
                    ets_all.append(ets)
                    st = [psA.tile([128, 1536], F32, name="st", tag="a")
                          for _ in range(2)]

                    # chunk-outer / head-inner so adjacent matmuls use
                    # disjoint PE row groups and stream concurrently
                    for c, col0, width, q0 in sp["parts"]:
                        for hh in range(2):
                            hr = hh * 64
                            if c == "glb":
                                lhs = kpair[hr:hr + 64, 0:G]
                                rows = 64
                            else:
                                s0 = (2 * (n - 1) + c) * 128
                                lhs = kpair[hr:hr + 64, s0:s0 + 128]
                                rows = 128
                            nc.tensor.matmul(
                                st[hh][0:rows, col0:col0 + width],
                                lhs,
                                qpair[hr:hr + 64,
                                      n * 256 + q0:n * 256 + q0 + width],
                                start=True, stop=True)

                    for hh in range(2):
                        c0e, c1e, rows = sp["exp"]
                        nc.scalar.activation(
                            ets[hh][0:rows, c0e:c1e],
                            st[hh][0:rows, c0e:c1e], EXP, scale=SCALE)
                        for m0, m1, mi, moff in sp["masks"]:
                            nc.vector.tensor_mul(
                                ets[hh][:, m0:m1],
                                ets[hh][:, m0:m1],
                                mk_sb[:, mi, moff:moff + m1 - m0])
                    if pending is not None:
                        emit_wo_half(pending[0], pending[1], hp)

                for hp in range(2):
                    # PV + normalize; the two heads share one psum bank
                    ot = psC.tile([128, 512], F32, name="ot", tag="c")
                    for hh in range(2):
                        h = hp * 2 + hh
                        ob = hh * 256
                        et = ets_all[hp][hh]
                        for i, (c, eoff, width, q0) in enumerate(sp["pv"]):
                            s_tile = 2 * (n - 1) + c
                            nc.tensor.matmul(
                                ot[0:128, ob + q0:ob + q0 + width],
                                vv[:, s_tile, h, :],
                                et[:, eoff:eoff + width],
                                start=(i == 0), stop=False)
                        goff = sp["glb_off"]
                        nc.tensor.matmul(
                            ot[0:128, ob:ob + 256],
                            vv[0:64, 0, h, :],
                            et[0:64, goff:goff + 256],
                            start=False, stop=True)
                    # reciprocal_approx_fast needs exact fp32 bits; its PSUM
                    # read path perturbs them (HW-measured ~5% error), so
                    # bounce the denominator row through SBUF.
                    row = smp.tile([1, 512], F32, name="row")
                    nc.vector.tensor_copy(row[:], ot[64:65, :])
                    den = smp.tile([1, 512], F32, name="den")
                    nc.vector.reciprocal_approx_fast(den[:], row[:])
                    recb = smp.tile([64, 512], F32, name="recb")
                    nc.gpsimd.partition_broadcast(recb[:], den[:])
                    for hh in range(2):
                        nc.vector.tensor_mul(
                            at_blk[hh * 64:(hh + 1) * 64, hp, :],
                            ot[0:64, hh * 256:(hh + 1) * 256],
                            recb[:, hh * 256:(hh + 1) * 256])

            # ---- software pipeline: proj chunks interleaved with attention
            # blocks whose q/k/v deps are ready (block n needs chunks
            # <= (n+1)//2); Wo for block n is emitted after block n+1's
            # attention so the in-order PE never stalls on the normalize.
            sched = [("p", 0), ("a", 0), ("p", 1), ("a", 1), ("a", 2),
                     ("p", 2), ("a", 3), ("a", 4), ("p", 3), ("a", 5),
                     ("a", 6), ("p", 4), ("a", 7), ("a", 8), ("p", 5),
                     ("a", 9), ("a", 10), ("p", 6), ("a", 11), ("a", 12),
                     ("p", 7), ("a", 13), ("a", 14), ("a", 15)]
            pending = None
            for kind, idx in sched:
                if kind == "p":
                    emit_proj(idx)
                else:
                    at_blk = atp.tile([128, 2, 256], BF16, name="at_blk")
                    emit_attn(idx, at_blk, pending)
                    pending = (idx, at_blk)
            for ss in range(2):
                emit_wo_half(pending[0], pending[1], ss)

    nc.compile()
    return nc


def _get_nc():
    if "nc" not in _BUILT:
        _BUILT["nc"] = build()
    return _BUILT["nc"]


def make_in_maps(x, Wq, Wk, Wv, Wo):
    masks_np = build_masks()
    xT = [np.ascontiguousarray(x[b].T).astype(NPBF) for b in range(B)]
    wq16, wk16, wv16 = (w.astype(NPBF) for w in (Wq, Wk, Wv))
    wo16 = Wo.astype(NPBF)
    in_maps = []
    for core in range(N_CORES):
        b, hg = core // 4, core % 4
        cols = slice(hg * ECOLS, (hg + 1) * ECOLS)
        in_maps.append({
            "xT": xT[b],
            "wq": np.ascontiguousarray(wq16[:, cols]),
            "wk": np.ascontiguousarray(wk16[:, cols]),
            "wv": np.ascontiguousarray(wv16[:, cols]),
            "wo": np.ascontiguousarray(wo16[cols, :]),
            "masks": masks_np,
        })
    return in_maps


def kernel(x, Wq, Wk, Wv, Wo):
    global LAST_RESULTS
    nc = _get_nc()
    in_maps = make_in_maps(x, Wq, Wk, Wv, Wo)
    res = run_bass_kernel_spmd(nc, in_maps, core_ids=list(range(N_CORES)))
    LAST_RESULTS = res
    out = np.zeros((B, S, D), np.float32)
    for core in range(N_CORES):
        out[core // 4] += res.results[core]["y"].astype(np.float32)
    return out
